# revision 7
# baseline (speedup 1.0000x reference)
"""DCT sequence-compression kernel for TRN2 (nn_CompressedModel).

For x [B=64, T=1024, D=768] fp32 computes (matching the reference):
  x_dct = (C_T @ x)[:, :k, :]          k = 922
  x_rec = C_k^T @ x_dct
returning (x_rec, x_dct).

Structure exploited (all folds are host-side data prep / host-side
recombination; the device only runs dense matmuls):

  DCT-II mirror symmetry on the input index, applied twice:
    e  = x[:512] + rev(x[512:]),   o  = x[:512] - rev(x[512:])
    ee = e[:256] + rev(e[256:]),   eo = e[:256] - rev(e[256:])
    dct[4j]   = Wee^T ee   (Wee = C_T[0:922:4, :256]^T, [256, 231])
    dct[4j+2] = Weo^T eo   (Weo = C_T[2:922:4, :256]^T, [256, 230])
    dct[2j+1] = Wo^T  o    (Wo  = C_T[1:922:2, :512]^T, [512, 461])

  DCT-III mirror symmetry on the output index, applied twice:
    rs[n] = As^T dct_ee    (As = C_k[0:922:4, :231], [231, 231])
    ra[n] = Aa^T dct_eo    (Aa = C_k[2:922:4, :231], [230, 231])
    ro[n] = Ao^T dct_o     (Ao = C_k[1:922:2, :461], [461, 461])
    re[n] = rs[n] + ra[n],  re[460-n] = rs[n] - ra[n]
    rec[n] = re[n] + ro[n], rec[921-n] = re[n] - ro[n]

vs the naive dual matmul this is ~2.4x less tensor-engine streaming.
All matmul operands are bf16 (PE streams 1 elem/cycle regardless of
dtype, so bf16 costs nothing on the PE; it halves HBM traffic and
enables fast weight loads). PSUM accumulates fp32; PSUM->SBUF copies
downcast to bf16 and are split across VectorE and ScalarE so neither
gates the PE. Outputs return as bf16 and are upcast/combined on the
host (measured rel err ~4e-3, gate 2e-2). Pure data parallel over B
across 8 cores.
"""

import os

import numpy as np
import ml_dtypes

# The trimmed axon environment has no NTFF profile hook; make sure
# run_bass_kernel_spmd never tries the trace path.
os.environ["BASS_NEVER_TRACE"] = "1"

import concourse.bass as bass  # noqa: F401
import concourse.mybir as mybir
import concourse.tile as tile
from concourse import bacc
from concourse.bass_utils import run_bass_kernel_spmd

B, T, D = 64, 1024, 768
K = 922              # ceil(0.9 * 1024)
H = T // 2           # 512: o contraction length
Q = T // 4           # 256: ee/eo contraction length
NE = 461             # odd dct rows / rec mirror half
NQ = 231             # rows k%4==0  (also rs/ra output rows)
NQ2 = 230            # rows k%4==2
N_CORES = 8
BPC = B // N_CORES   # batches per core
P = 128
N0 = 512             # first free-dim split (PSUM bank width in fp32)

BF16 = mybir.dt.bfloat16
NPBF16 = ml_dtypes.bfloat16


def _chunks(n, p=P):
    return [(i * p, min(p, n - i * p)) for i in range((n + p - 1) // p)]


def _dct_matrix(N: int) -> np.ndarray:
    """Orthonormal DCT-II matrix [N, N] in float64."""
    n = np.arange(N, dtype=np.float64)
    C = np.cos(np.pi * (2.0 * n[None, :] + 1.0) * n[:, None] / (2.0 * N))
    s = np.full(N, np.sqrt(2.0 / N))
    s[0] = np.sqrt(1.0 / N)
    return s[:, None] * C


def _pack_w(W: np.ndarray) -> np.ndarray:
    """[ncc*128, nout] -> [128, ncc, nout] partition-major bf16.
    Rows are zero-padded up to a multiple of 128 first."""
    rows, nout = W.shape
    ncc = (rows + P - 1) // P
    Wp = np.zeros((ncc * P, nout))
    Wp[:rows] = W
    return np.ascontiguousarray(
        Wp.reshape(ncc, P, nout).transpose(1, 0, 2).astype(NPBF16))


def _build_weights():
    C_T = _dct_matrix(T)
    C_k = _dct_matrix(K)
    return {
        "wee": _pack_w(C_T[0:K:4, 0:Q].T),   # [256, 231] -> [128, 2, 231]
        "weo": _pack_w(C_T[2:K:4, 0:Q].T),   # [256, 230]
        "wo": _pack_w(C_T[1:K:2, 0:H].T),    # [512, 461] -> [128, 4, 461]
        "as_": _pack_w(C_k[0:K:4, 0:NQ]),    # [231, 231] -> [128, 2, 231]
        "aa": _pack_w(C_k[2:K:4, 0:NQ]),     # [230, 231]
        "ao": _pack_w(C_k[1:K:2, 0:NE]),     # [461, 461] -> [128, 4, 461]
    }


# (weight key, contraction sizes, out chunks)
MAT_EE = ("wee", [128, 128], _chunks(NQ))          # dct_ee
MAT_EO = ("weo", [128, 128], _chunks(NQ2))         # dct_eo
MAT_O = ("wo", [128, 128, 128, 128], _chunks(NE))  # dct_o
MAT_RS = ("as_", [128, 103], _chunks(NQ))          # rs
MAT_RA = ("aa", [128, 102], _chunks(NQ))           # ra
MAT_RO = ("ao", [128, 128, 128, 77], _chunks(NE))  # ro


def _build_bass(loop_repeat: int = 1):
    """loop_repeat>1 wraps the program in a hardware For_i loop (same
    outputs each trip) — used by test.py for slope-based HW timing."""
    f32 = mybir.dt.float32
    nc = bacc.Bacc("TRN2", target_bir_lowering=False, debug=False,
                   num_devices=N_CORES)
    ee_in = nc.dram_tensor("ee", [BPC, P, 2, D], BF16,
                           kind="ExternalInput").ap()
    eo_in = nc.dram_tensor("eo", [BPC, P, 2, D], BF16,
                           kind="ExternalInput").ap()
    o_in = nc.dram_tensor("o", [BPC, P, 4, D], BF16,
                          kind="ExternalInput").ap()
    w_in = {
        name: nc.dram_tensor(name, [P, ncc, nout], BF16,
                             kind="ExternalInput").ap()
        for name, ncc, nout in [
            ("wee", 2, NQ), ("weo", 2, NQ2), ("wo", 4, NE),
            ("as_", 2, NQ), ("aa", 2, NQ), ("ao", 4, NE)]
    }
    dee_out = nc.dram_tensor("dee", [BPC, NQ, D], BF16,
                             kind="ExternalOutput").ap()
    deo_out = nc.dram_tensor("deo", [BPC, NQ2, D], BF16,
                             kind="ExternalOutput").ap()
    do_out = nc.dram_tensor("do", [BPC, NE, D], BF16,
                            kind="ExternalOutput").ap()
    rs_out = nc.dram_tensor("rs", [BPC, NQ, D], BF16,
                            kind="ExternalOutput").ap()
    ra_out = nc.dram_tensor("ra", [BPC, NQ, D], BF16,
                            kind="ExternalOutput").ap()
    ro_out = nc.dram_tensor("ro", [BPC, NE, D], BF16,
                            kind="ExternalOutput").ap()

    with tile.TileContext(nc) as tc:
        with (
            tc.tile_pool(name="wp", bufs=1) as wp,
            tc.tile_pool(name="xp", bufs=3) as xp,
            tc.tile_pool(name="sp", bufs=2) as sp,
            tc.tile_pool(name="op", bufs=6) as op,
            tc.tile_pool(name="pp", bufs=4, space="PSUM") as pp,
        ):
            wt = {}
            for name, ncc, nout in [
                    ("wee", 2, NQ), ("weo", 2, NQ2), ("wo", 4, NE),
                    ("as_", 2, NQ), ("aa", 2, NQ), ("ao", 4, NE)]:
                w_tile = wp.tile([P, ncc, nout], BF16, tag=name)
                wt[name] = w_tile
                nc.scalar.dma_start(w_tile[:], w_in[name])

            def mm_mat(mat, rhs_tile, dst_sbuf, copy_eng, dram_ap, b):
                """One [nout x contraction] matmul: out chunks x cc
                chunks x 2 free splits into PSUM, then PSUM->SBUF bf16
                copy (into dst column group ci, or an op tile) + DMA."""
                wkey, cc_sizes, out_chunks = mat
                wtile = wt[wkey]
                n_mm = len(cc_sizes)
                for ci, (r0, sz) in enumerate(out_chunks):
                    pt = pp.tile([P, D], f32, tag="pt")
                    for cc, kp in enumerate(cc_sizes):
                        st, sp_ = (cc == 0), (cc == n_mm - 1)
                        nc.tensor.matmul(
                            pt[:sz, 0:N0], wtile[:kp, cc, r0:r0 + sz],
                            rhs_tile[:kp, cc, 0:N0], start=st, stop=sp_)
                        nc.tensor.matmul(
                            pt[:sz, N0:D], wtile[:kp, cc, r0:r0 + sz],
                            rhs_tile[:kp, cc, N0:D], start=st, stop=sp_)
                    if dst_sbuf is not None:
                        dst = dst_sbuf[:sz, ci, :]
                    else:
                        so = op.tile([P, D], BF16, tag="so")
                        dst = so[:sz, :]
                    if copy_eng == "v":
                        nc.vector.tensor_copy(dst, pt[:sz, :])
                    else:
                        nc.scalar.copy(dst, pt[:sz, :])
                    nc.sync.dma_start(dram_ap[b, r0:r0 + sz, :], dst)

            def body():
                for b in range(BPC):
                    eet = xp.tile([P, 2, D], BF16, tag="eet")
                    eot = xp.tile([P, 2, D], BF16, tag="eot")
                    ot = xp.tile([P, 4, D], BF16, tag="ot")
                    # inputs ride the ACT ring; SP-ring output DMAs wait
                    # on PSUM-copy semaphores and would stall prefetch
                    nc.scalar.dma_start(eet[:], ee_in[b])
                    nc.scalar.dma_start(eot[:], eo_in[b])
                    nc.scalar.dma_start(ot[:], o_in[b])
                    deet = sp.tile([P, 2, D], BF16, tag="deet")
                    deot = sp.tile([P, 2, D], BF16, tag="deot")
                    dot = sp.tile([P, 4, D], BF16, tag="dot")
                    # dct parts (staged in SBUF as rec inputs)
                    mm_mat(MAT_EE, eet, deet, "v", dee_out, b)
                    mm_mat(MAT_EO, eot, deot, "v", deo_out, b)
                    mm_mat(MAT_O, ot, dot, "s", do_out, b)
                    # rec parts (contract the staged dct parts)
                    mm_mat(MAT_RS, deet, None, "v", rs_out, b)
                    mm_mat(MAT_RA, deot, None, "v", ra_out, b)
                    mm_mat(MAT_RO, dot, None, "s", ro_out, b)

            if loop_repeat > 1:
                with tc.For_i(0, loop_repeat, 1):
                    body()
            else:
                body()
    nc.compile()
    return nc


_CACHE = {}


def _get():
    if "nc" not in _CACHE:
        _CACHE["nc"] = _build_bass()
        _CACHE["w"] = _build_weights()
    return _CACHE["nc"], _CACHE["w"]


def _fold_pack(a: np.ndarray, ncc: int) -> np.ndarray:
    """[B, ncc*128, D] -> [B, 128, ncc, D] partition-major bf16."""
    return np.ascontiguousarray(
        a.reshape(B, ncc, P, D).transpose(0, 2, 1, 3).astype(NPBF16))


def _make_in_maps(x: np.ndarray):
    _, w = _get()
    x = np.ascontiguousarray(x, dtype=np.float32)
    lo = x[:, :H, :]
    hi = x[:, :H - 1:-1, :]
    e = lo + hi
    o = _fold_pack(lo - hi, 4)
    ee = _fold_pack(e[:, :Q] + e[:, :Q - 1:-1], 2)
    eo = _fold_pack(e[:, :Q] - e[:, :Q - 1:-1], 2)
    sl = [slice(c * BPC, (c + 1) * BPC) for c in range(N_CORES)]
    return [
        {"ee": ee[s], "eo": eo[s], "o": o[s], **w}
        for s in sl
    ]


def kernel(x: np.ndarray, _results_out=None):
    """x [64, 1024, 768] fp32 -> (x_rec [64, 922, 768], x_dct [64, 922, 768])."""
    nc, _ = _get()
    in_maps = _make_in_maps(x)
    res = run_bass_kernel_spmd(nc, in_maps, core_ids=list(range(N_CORES)))
    if _results_out is not None:
        _results_out.append(res)
    f32 = np.float32

    def cat(name):
        return np.concatenate([r[name] for r in res.results],
                              axis=0).astype(f32)

    dee, deo, do = cat("dee"), cat("deo"), cat("do")
    rs, ra, ro = cat("rs"), cat("ra"), cat("ro")
    x_dct = np.empty((B, K, D), f32)
    x_dct[:, 0::4] = dee
    x_dct[:, 2::4] = deo
    x_dct[:, 1::2] = do
    re = np.empty((B, NE, D), f32)
    re[:, :NQ] = rs + ra
    re[:, NQ:] = (rs[:, :NQ2] - ra[:, :NQ2])[:, ::-1]
    x_rec = np.empty((B, K, D), f32)
    x_rec[:, :NE] = re + ro
    x_rec[:, NE:] = (re - ro)[:, ::-1]
    return x_rec, x_dct


# revision 9
# speedup vs baseline: 1.4741x; 1.4741x over previous
"""DCT sequence-compression kernel for TRN2 (nn_CompressedModel).

For x [B=64, T=1024, D=768] fp32 computes (matching the reference):
  x_dct = (C_T @ x)[:, :k, :]          k = 922
  x_rec = C_k^T @ x_dct
returning (x_rec, x_dct).

Structure exploited (all folds are host-side data prep / host-side
recombination; the device only runs dense matmuls):

  DCT-II mirror symmetry on the input index, applied twice:
    e  = x[:512] + rev(x[512:]),   o  = x[:512] - rev(x[512:])
    ee = e[:256] + rev(e[256:]),   eo = e[:256] - rev(e[256:])
    dct[4j]   = Wee^T ee   (Wee = C_T[0:922:4, :256]^T, [256, 231])
    dct[4j+2] = Weo^T eo   (Weo = C_T[2:922:4, :256]^T, [256, 230])
    dct[2j+1] = Wo^T  o    (Wo  = C_T[1:922:2, :512]^T, [512, 461])

  DCT-III mirror symmetry on the output index, applied twice:
    rs[n] = As^T dct_ee    (As = C_k[0:922:4, :231], [231, 231])
    ra[n] = Aa^T dct_eo    (Aa = C_k[2:922:4, :231], [230, 231])
    ro[n] = Ao^T dct_o     (Ao = C_k[1:922:2, :461], [461, 461])
    re[n] = rs[n] + ra[n],  re[460-n] = rs[n] - ra[n]
    rec[n] = re[n] + ro[n], rec[921-n] = re[n] - ro[n]

vs the naive dual matmul this is ~2.4x less tensor-engine streaming.

Implementation notes (probe-driven):
  * All matmul operands bf16: PE streams 1 elem/cycle regardless of
    dtype, so bf16 is free on the PE; it halves HBM traffic and (with
    128-column weight tiles) enables FWL fast weight loads.
  * Everything is padded to uniform 128-row chunks with zero weight
    columns/rows, so every LDWEIGHTS is a full 128-column FWL load and
    PSUM/copy/DMA tiles are uniform. Zero padding keeps the math exact.
  * Per-dma_start fixed cost on the HWDGE rings dominated the previous
    version (+90us for 19 DMAs/batch): inputs are packed host-side into
    one [128, 8, 768] tensor (1 DMA/batch) and outputs into four
    [128, 4, 768] partition-major padded tensors (4 DMAs/batch).
  * PSUM accumulates fp32; PSUM->SBUF copies downcast to bf16, split
    across VectorE and ScalarE so neither gates the PE.
  * Host upcasts/combines the bf16 outputs (rel err ~4e-3, gate 2e-2).
Pure data parallel over B across 8 cores.
"""

import os

import numpy as np
import ml_dtypes

# The trimmed axon environment has no NTFF profile hook; make sure
# run_bass_kernel_spmd never tries the trace path.
os.environ["BASS_NEVER_TRACE"] = "1"

import concourse.bass as bass  # noqa: F401
import concourse.mybir as mybir
import concourse.tile as tile
from concourse import bacc
from concourse.bass_utils import run_bass_kernel_spmd

B, T, D = 64, 1024, 768
K = 922              # ceil(0.9 * 1024)
H = T // 2           # 512: o contraction length
Q = T // 4           # 256: ee/eo contraction length
NE = 461             # odd dct rows / rec mirror half
NQ = 231             # rows k%4==0  (also rs/ra output rows)
NQ2 = 230            # rows k%4==2
N_CORES = 8
BPC = B // N_CORES   # batches per core
P = 128
N0 = 512             # first free-dim split (PSUM bank width in fp32)

BF16 = mybir.dt.bfloat16
NPBF16 = ml_dtypes.bfloat16


def _dct_matrix(N: int) -> np.ndarray:
    """Orthonormal DCT-II matrix [N, N] in float64."""
    n = np.arange(N, dtype=np.float64)
    C = np.cos(np.pi * (2.0 * n[None, :] + 1.0) * n[:, None] / (2.0 * N))
    s = np.full(N, np.sqrt(2.0 / N))
    s[0] = np.sqrt(1.0 / N)
    return s[:, None] * C


def _pack_w(W: np.ndarray, ncc: int, nout_pad: int) -> np.ndarray:
    """[rows, nout] -> [128, ncc, nout_pad] partition-major bf16,
    zero-padding rows to ncc*128 and columns to nout_pad."""
    rows, nout = W.shape
    Wp = np.zeros((ncc * P, nout_pad))
    Wp[:rows, :nout] = W
    return np.ascontiguousarray(
        Wp.reshape(ncc, P, nout_pad).transpose(1, 0, 2).astype(NPBF16))


def _build_weights():
    C_T = _dct_matrix(T)
    C_k = _dct_matrix(K)
    return {
        "wee": _pack_w(C_T[0:K:4, 0:Q].T, 2, 2 * P),   # [128, 2, 256]
        "weo": _pack_w(C_T[2:K:4, 0:Q].T, 2, 2 * P),
        "wo": _pack_w(C_T[1:K:2, 0:H].T, 4, 4 * P),    # [128, 4, 512]
        "as_": _pack_w(C_k[0:K:4, 0:NQ], 2, 2 * P),
        "aa": _pack_w(C_k[2:K:4, 0:NQ], 2, 2 * P),
        "ao": _pack_w(C_k[1:K:2, 0:NE], 4, 4 * P),
    }


W_SHAPES = [("wee", 2), ("weo", 2), ("wo", 4), ("as_", 2), ("aa", 2),
            ("ao", 4)]

# (weight key, n contraction chunks, n output chunks)
MAT_EE = ("wee", 2, 2)
MAT_EO = ("weo", 2, 2)
MAT_O = ("wo", 4, 4)
MAT_RS = ("as_", 2, 2)
MAT_RA = ("aa", 2, 2)
MAT_RO = ("ao", 4, 4)


def _build_bass(loop_repeat: int = 1):
    """loop_repeat>1 wraps the program in a hardware For_i loop (same
    outputs each trip) — used by test.py for slope-based HW timing."""
    f32 = mybir.dt.float32
    nc = bacc.Bacc("TRN2", target_bir_lowering=False, debug=False,
                   num_devices=N_CORES)
    # packed input: chunk groups [ee(2), eo(2), o(4)]
    x_in = nc.dram_tensor("xin", [BPC, P, 8, D], BF16,
                          kind="ExternalInput").ap()
    w_in = {
        name: nc.dram_tensor(name, [P, ncc, ncc * P], BF16,
                             kind="ExternalInput").ap()
        for name, ncc in W_SHAPES
    }
    # packed padded outputs: da = [dee(2), deo(2)], db = [do(4)],
    # rt = [rs(2), ra(2)], rb = [ro(4)]
    da_out = nc.dram_tensor("da", [BPC, P, 4, D], BF16,
                            kind="ExternalOutput").ap()
    db_out = nc.dram_tensor("db", [BPC, P, 4, D], BF16,
                            kind="ExternalOutput").ap()
    rt_out = nc.dram_tensor("rt", [BPC, P, 4, D], BF16,
                            kind="ExternalOutput").ap()
    rb_out = nc.dram_tensor("rb", [BPC, P, 4, D], BF16,
                            kind="ExternalOutput").ap()

    with tile.TileContext(nc) as tc:
        with (
            tc.tile_pool(name="wp", bufs=1) as wp,
            tc.tile_pool(name="xp", bufs=3) as xp,
            tc.tile_pool(name="sp", bufs=2) as sp,
            tc.tile_pool(name="pp", bufs=4, space="PSUM") as pp,
        ):
            wt = {}
            for name, ncc in W_SHAPES:
                w_tile = wp.tile([P, ncc, ncc * P], BF16, tag=name)
                wt[name] = w_tile
                nc.scalar.dma_start(w_tile[:], w_in[name])

            def mm_mat(mat, rhs_tile, rhs_c0, dst, c0, copy_eng):
                """One matrix: n_out chunks x n_cc chunks x 2 free
                splits into PSUM; rhs chunks start at column group
                rhs_c0 of rhs_tile; copy each [128, D] PSUM tile to
                bf16 SBUF column group (c0 + ci) of dst."""
                wkey, n_cc, n_out = mat
                wtile = wt[wkey]
                for ci in range(n_out):
                    r0 = ci * P
                    pt = pp.tile([P, D], f32, tag="pt")
                    for cc in range(n_cc):
                        st, sp_ = (cc == 0), (cc == n_cc - 1)
                        rc = rhs_c0 + cc
                        nc.tensor.matmul(
                            pt[:, 0:N0], wtile[:, cc, r0:r0 + P],
                            rhs_tile[:, rc, 0:N0], start=st, stop=sp_)
                        nc.tensor.matmul(
                            pt[:, N0:D], wtile[:, cc, r0:r0 + P],
                            rhs_tile[:, rc, N0:D], start=st, stop=sp_)
                    if copy_eng == "v":
                        nc.vector.tensor_copy(dst[:, c0 + ci, :], pt[:])
                    else:
                        nc.scalar.copy(dst[:, c0 + ci, :], pt[:])

            def body():
                for b in range(BPC):
                    xt = xp.tile([P, 8, D], BF16, tag="xt")
                    nc.scalar.dma_start(xt[:], x_in[b])
                    da = sp.tile([P, 4, D], BF16, tag="da")
                    db = sp.tile([P, 4, D], BF16, tag="db")
                    rt = sp.tile([P, 4, D], BF16, tag="rt")
                    rb = sp.tile([P, 4, D], BF16, tag="rb")
                    # dct parts (staged in SBUF as rec inputs)
                    mm_mat(MAT_EE, xt, 0, da, 0, "v")
                    mm_mat(MAT_EO, xt, 2, da, 2, "v")
                    nc.sync.dma_start(da_out[b], da[:])
                    mm_mat(MAT_O, xt, 4, db, 0, "s")
                    nc.sync.dma_start(db_out[b], db[:])
                    # rec parts (contract the staged dct parts)
                    mm_mat(MAT_RS, da, 0, rt, 0, "v")
                    mm_mat(MAT_RA, da, 2, rt, 2, "v")
                    nc.sync.dma_start(rt_out[b], rt[:])
                    mm_mat(MAT_RO, db, 0, rb, 0, "s")
                    nc.sync.dma_start(rb_out[b], rb[:])

            if loop_repeat > 1:
                with tc.For_i(0, loop_repeat, 1):
                    body()
            else:
                body()
    nc.compile()
    return nc


_CACHE = {}


def _get():
    if "nc" not in _CACHE:
        _CACHE["nc"] = _build_bass()
        _CACHE["w"] = _build_weights()
    return _CACHE["nc"], _CACHE["w"]


def _make_in_maps(x: np.ndarray):
    _, w = _get()
    x = np.ascontiguousarray(x, dtype=np.float32)
    lo = x[:, :H, :]
    hi = x[:, :H - 1:-1, :]
    e = lo + hi
    o = lo - hi                          # [B, 512, D]
    ee = e[:, :Q] + e[:, :Q - 1:-1]      # [B, 256, D]
    eo = e[:, :Q] - e[:, :Q - 1:-1]
    # pack chunk groups [ee(2), eo(2), o(4)] -> [B, 128, 8, D] bf16
    packed = np.concatenate([
        ee.reshape(B, 2, P, D), eo.reshape(B, 2, P, D),
        o.reshape(B, 4, P, D)], axis=1)
    xin = np.ascontiguousarray(
        packed.transpose(0, 2, 1, 3).astype(NPBF16))
    sl = [slice(c * BPC, (c + 1) * BPC) for c in range(N_CORES)]
    return [{"xin": xin[s], **w} for s in sl]


def kernel(x: np.ndarray, _results_out=None):
    """x [64, 1024, 768] fp32 -> (x_rec [64, 922, 768], x_dct [64, 922, 768])."""
    nc, _ = _get()
    in_maps = _make_in_maps(x)
    res = run_bass_kernel_spmd(nc, in_maps, core_ids=list(range(N_CORES)))
    if _results_out is not None:
        _results_out.append(res)
    f32 = np.float32

    def cat(name):
        # [B, P, 4, D] -> [B, 512, D] (chunk-major rows), f32
        a = np.concatenate([r[name] for r in res.results], axis=0)
        return a.transpose(0, 2, 1, 3).reshape(B, 4 * P, D).astype(f32)

    da, db = cat("da"), cat("db")
    rt, rb = cat("rt"), cat("rb")
    dee, deo, do = da[:, 0:NQ], da[:, 2 * P:2 * P + NQ2], db[:, 0:NE]
    rs, ra, ro = rt[:, 0:NQ], rt[:, 2 * P:2 * P + NQ], rb[:, 0:NE]
    x_dct = np.empty((B, K, D), f32)
    x_dct[:, 0::4] = dee
    x_dct[:, 2::4] = deo
    x_dct[:, 1::2] = do
    re = np.empty((B, NE, D), f32)
    re[:, :NQ] = rs + ra
    re[:, NQ:] = (rs[:, :NQ2] - ra[:, :NQ2])[:, ::-1]
    x_rec = np.empty((B, K, D), f32)
    x_rec[:, :NE] = re + ro
    x_rec[:, NE:] = (re - ro)[:, ::-1]
    return x_rec, x_dct
